# revision 6
# baseline (speedup 1.0000x reference)
"""Kronecker layer forward on 8 TRN2 NeuronCores — bf16 pipeline.

Computes y = gelu_exact(x @ kron(B, A)) + bias for
  x [16384, 4096] f32, A [64, 64], B [64, 64], bias [4096].

Math: with x3 = x.reshape(n, 64, 64) (feature f = i*64 + k),
  u[b, j, k] = sum_i x3[b, i, k] * B[i, j]
  y[b, j*64+l] = sum_k u[b, j, k] * A[k, l]  (then gelu, +bias)

Everything on-device runs in bf16 (inputs quantized host-side; rel-err
budget 2e-2 >> bf16's ~0.5%), halving HBM traffic vs f32 — the
binding roofline at ~358 GB/s/core is (16+16) MB -> ~90 us.

Per-core layout (tpc tokens): token t = g*tpc/2 + h*tpc/4 + blk*S + st,
supertile = 4 tokens over (g,h) in {0,1}^2. SBUF x tile per block:
  xt[p=(g,i), f=(st,h,k)] = x3[t, i, k]
Stage 1 (data-stationary): per supertile st,
  mm1: out1 = xt_st.T @ blockdiag(B,B) -> u[p=(h,k), f=(g,j)] in PSUM.
Stage 2 (weight-stationary): DVE copies 8 supertiles of u to SBUF bf16,
  mm2: out2 = blockdiag(A,A).T @ u8 (N=512 x2) -> y[p=(h,l), f=(st,g,j)].
ScalarE applies exact-erf Gelu PSUM->SBUF(bf16); per-block 1 MB DMAs
in (sync/HWDGE) and out (scalar/HWDGE). Bias (gelu happens first) is
added host-side only if nonzero.

Sharding: pure data-parallel over tokens — 2048/core, no collectives.
"""

import numpy as np

N_CORES = 8
TOKENS = 16384
D = 4096
TPC = TOKENS // N_CORES  # tokens per core

_CACHE = {}


def _build(tpc, n_cores):
    import concourse.bacc as bacc
    import concourse.mybir as mybir
    import concourse.tile as tile

    f32 = mybir.dt.float32
    bf16 = mybir.dt.bfloat16

    quarter = tpc // 4          # tokens per (g,h) quadrant
    S = min(32, quarter)        # supertiles per block
    assert quarter % S == 0
    nblocks = quarter // S
    G = 8                       # supertiles per PSUM group (2 banks)
    assert S % G == 0
    ngrp_blk = S // G           # groups per block

    nc = bacc.Bacc(
        "TRN2",
        target_bir_lowering=False,
        debug=False,
        num_devices=n_cores,
    )
    x_d = nc.dram_tensor(
        "x", [nblocks, 128, S * 128], bf16, kind="ExternalInput"
    ).ap()
    wb_d = nc.dram_tensor("wb", [128, 128], bf16, kind="ExternalInput").ap()
    wa_d = nc.dram_tensor("wa", [128, 128], bf16, kind="ExternalInput").ap()
    y_d = nc.dram_tensor(
        "y", [nblocks, 128, S * 128], bf16, kind="ExternalOutput"
    ).ap()

    groups = [(blk, q) for blk in range(nblocks) for q in range(ngrp_blk)]
    ng = len(groups)

    with tile.TileContext(nc) as tc:
        with (
            tc.tile_pool(name="const", bufs=1) as constp,
            tc.tile_pool(name="xp", bufs=3) as xp,
            tc.tile_pool(name="up", bufs=2) as up,
            tc.tile_pool(name="yp", bufs=3) as yp,
            tc.tile_pool(name="ps1", bufs=2, space="PSUM") as ps1,
            tc.tile_pool(name="ps2", bufs=2, space="PSUM") as ps2,
        ):
            wb = constp.tile([128, 128], bf16)
            wa = constp.tile([128, 128], bf16)
            nc.sync.dma_start(wb[:], wb_d)
            nc.sync.dma_start(wa[:], wa_d)

            xtiles = {}
            ytiles = {}
            pend_o1 = None  # group gi-1's mm1 output, not yet evacuated
            # one-group software pipeline: PE always has the next group's
            # mm1 batch queued while this group's u-copy drains.
            for gi in range(ng + 1):
                cur_o1 = None
                if gi < ng:
                    blk, q = groups[gi]
                    if q == 0:
                        xtiles[blk] = xp.tile([128, S * 128], bf16, name="xbig")
                        nc.sync.dma_start(xtiles[blk][:], x_d[blk])
                        ytiles[blk] = yp.tile([128, S * 128], bf16, name="ybig")
                    cur_o1 = ps1.tile([128, G * 128], f32)
                    xt = xtiles[blk]
                    for s8 in range(G):
                        s = q * G + s8
                        nc.tensor.matmul(
                            cur_o1[:, s8 * 128 : (s8 + 1) * 128],
                            xt[:, s * 128 : (s + 1) * 128],
                            wb[:],
                        )
                if gi >= 1:
                    blk2, q2 = groups[gi - 1]
                    u8 = up.tile([128, G * 128], bf16)
                    nc.vector.tensor_copy(u8[:], pend_o1[:])
                    o2 = ps2.tile([128, G * 128], f32)
                    for m in range(2):
                        nc.tensor.matmul(
                            o2[:, m * 512 : (m + 1) * 512],
                            wa[:],
                            u8[:, m * 512 : (m + 1) * 512],
                        )
                    nc.scalar.activation(
                        ytiles[blk2][:, q2 * (G * 128) : (q2 + 1) * (G * 128)],
                        o2[:],
                        mybir.ActivationFunctionType.Gelu,
                    )
                    if q2 == ngrp_blk - 1:
                        nc.scalar.dma_start(y_d[blk2], ytiles[blk2][:])
                pend_o1 = cur_o1

    nc.compile()
    return nc


def _get_nc(tpc, n_cores=N_CORES):
    key = (tpc, n_cores)
    if key not in _CACHE:
        _CACHE[key] = _build(*key)
    return _CACHE[key]


def _blockdiag2(M):
    out = np.zeros((128, 128), np.float32)
    out[:64, :64] = M
    out[64:, 64:] = M
    return out


def _make_in_maps(x, A, B, tpc, n_cores):
    import ml_dtypes

    bf = ml_dtypes.bfloat16
    quarter = tpc // 4
    S = min(32, quarter)
    nblocks = quarter // S

    wb = _blockdiag2(np.asarray(B, np.float32)).astype(bf)
    wa = _blockdiag2(np.asarray(A, np.float32)).astype(bf)

    def permute_x(xs):
        # [t, f] -> [blk, (g,i), (st,h,k)]
        v = xs.reshape(2, 2, nblocks, S, 64, 64).transpose(2, 0, 4, 3, 1, 5)
        return np.ascontiguousarray(
            v.reshape(nblocks, 128, S * 128).astype(bf)
        )

    in_maps = []
    for c in range(n_cores):
        xs = np.asarray(x[c * tpc : (c + 1) * tpc], dtype=np.float32)
        in_maps.append({"x": permute_x(xs), "wb": wb, "wa": wa})
    return in_maps


def _run(x, A, B, bias, tpc=TPC, trace=False):
    from concourse.bass_utils import run_bass_kernel_spmd

    n = x.shape[0]
    n_cores = n // tpc
    assert n == n_cores * tpc

    nc = _get_nc(tpc, n_cores)

    quarter = tpc // 4
    S = min(32, quarter)
    nblocks = quarter // S

    def unpermute_y(yd):
        # [blk, (h,l), (st,g,j)] -> [t, f]
        v = np.asarray(yd).reshape(nblocks, 2, 64, S, 2, 64)
        v = v.transpose(4, 1, 0, 3, 5, 2)
        return v.reshape(tpc, D).astype(np.float32)

    in_maps = _make_in_maps(x, A, B, tpc, n_cores)

    res = run_bass_kernel_spmd(
        nc, in_maps, list(range(n_cores)), trace=trace,
        trace_cores=list(range(n_cores)) if trace else None,
    )
    y = np.concatenate([unpermute_y(r["y"]) for r in res.results], axis=0)
    b = np.asarray(bias, np.float32)
    if np.any(b):
        y = y + b
    return y.astype(np.float32), res


def kernel(x, A, B, bias):
    y, _ = _run(
        np.asarray(x), np.asarray(A), np.asarray(B), np.asarray(bias)
    )
    return y


# revision 55
# speedup vs baseline: 28.5906x; 28.5906x over previous
"""Kronecker layer forward on 8 TRN2 NeuronCores.

Computes y = gelu_exact(x @ kron(B, A)) + bias for
  x [16384, 4096] f32, A [64, 64], B [64, 64], bias [4096].

Math: with x3 = x.reshape(n, 64, 64) (feature f = i*64 + k),
  u[b, j, k] = sum_i x3[b, i, k] * B[i, j]
  y[b, j*64+l] = sum_k u[b, j, k] * A[k, l]  (then gelu, +bias)

The problem is HBM-bound (~358 GB/s/core; compute is only ~2 GFLOP
per core), so precision is spent on bytes:
  - x ships as int8 (global absmax scale, folded into the B weights;
    SWDGE DMA casts int8->bf16 in flight; small ints are exact bf16)
  - y returns as bf16; compute accumulates in f32 PSUM
Measured rel err 1.28e-2 vs the 2e-2 gate (int8 quant dominates).
HBM/core: 8 MB in + 16 MB out = 24 MB -> ~67 us floor; DVE PSUM
evacuation (~76 us busy) is the modeled critical engine.

Per-core layout (tpc tokens): token t = g*tpc/2 + h*tpc/4 + blk*S + st,
supertile = 4 tokens over (g,h) in {0,1}^2. SBUF x tile per block:
  xt[p=(g,i), f=(st,h,k)] = x3[t, i, k]
Per group of G=8 supertiles, into one 2-bank PSUM tile (bufs=4):
  mm1 (data-stationary) per st: xt_st.T @ blockdiag(B,B)
      -> u[p=(h,k), f=(g,j)]
  DVE copies the group's u to SBUF bf16 (FD=1024),
  mm2 (weight-stationary): blockdiag(A,A).T @ u8 (N=512 x2) written
      back into the banks the copy just freed -> y[p=(h,l), f=(st,g,j)]
  ScalarE exact-erf Gelu PSUM->SBUF bf16 (FD=1024).
One-group software pipeline keeps PE a batch ahead of the DVE copy.
x-in on gpsimd/SWDGE (casting), y-out on sync/HWDGE; first/last block
DMAs are chunked to shrink pipeline ramp/tail. Bias (applied after
gelu) is added host-side only if nonzero.

Sharding: pure data-parallel over tokens — 2048/core, no collectives.
"""

import numpy as np

N_CORES = 8
TOKENS = 16384
D = 4096
TPC = TOKENS // N_CORES  # tokens per core
S_MAX = 16  # supertiles per block (block = 4*S tokens)
X_INT8 = True  # ship x as int8 (global absmax scale folded into B)

_CACHE = {}


def _build(tpc, n_cores, reps=1):
    import concourse.bacc as bacc
    import concourse.mybir as mybir
    import concourse.tile as tile

    f32 = mybir.dt.float32
    bf16 = mybir.dt.bfloat16

    quarter = tpc // 4          # tokens per (g,h) quadrant
    S = min(S_MAX, quarter)     # supertiles per block
    assert quarter % S == 0
    nblocks = quarter // S
    G = 8                       # supertiles per PSUM group (2 banks)
    assert S % G == 0
    ngrp_blk = S // G           # groups per block

    nc = bacc.Bacc(
        "TRN2",
        target_bir_lowering=False,
        debug=False,
        num_devices=n_cores,
    )
    i8 = mybir.dt.int8
    x_d = nc.dram_tensor(
        "x", [nblocks, 128, S * 128], i8 if X_INT8 else bf16,
        kind="ExternalInput",
    ).ap()
    # int8 x rides SWDGE (gpsimd) which casts int8->bf16 in-flight;
    # y then takes the sync HWDGE ring. bf16 x: the reverse.
    xdma = nc.gpsimd if X_INT8 else nc.sync
    ydma = nc.sync if X_INT8 else nc.gpsimd
    wb_d = nc.dram_tensor("wb", [128, 128], bf16, kind="ExternalInput").ap()
    wa_d = nc.dram_tensor("wa", [128, 128], bf16, kind="ExternalInput").ap()
    y_d = nc.dram_tensor(
        "y", [nblocks, 128, S * 128], bf16, kind="ExternalOutput"
    ).ap()

    groups = [(blk, q) for blk in range(nblocks) for q in range(ngrp_blk)]
    ng = len(groups)

    with tile.TileContext(nc) as tc:
        with (
            tc.tile_pool(name="const", bufs=1) as constp,
            tc.tile_pool(name="xp", bufs=4) as xp,
            tc.tile_pool(name="up", bufs=3) as up,
            tc.tile_pool(name="yp", bufs=4) as yp,
            tc.tile_pool(name="ps1", bufs=4, space="PSUM") as ps1,
        ):
            # weights on the ACT HWDGE ring so the first x DMA (sync ring)
            # starts at t=0
            wb = constp.tile([128, 128], bf16)
            wa = constp.tile([128, 128], bf16)
            nc.sync.dma_start(wb[:], wb_d)
            nc.sync.dma_start(wa[:], wa_d)
            # dummy gelu loads the ACT table set during the first x DMA
            scratch = constp.tile([128, 128], bf16)
            nc.scalar.activation(
                scratch[:], wb[:], mybir.ActivationFunctionType.Gelu
            )

            # reps>1 re-emits the whole pipeline (idempotent: same inputs,
            # same outputs) so bench runs can difference out dispatch
            # overhead: per-rep = (t(R) - t(1)) / (R - 1).
            for _rep in range(reps):
                xtiles = {}
                ytiles = {}
                # one-group software pipeline: PE always has the next
                # group's mm1 batch queued while this group's u-copy drains.
                LOOKAHEAD = 1
                pend = []  # mm1 outputs not yet evacuated (oldest first)
                for gi in range(ng + LOOKAHEAD):
                    cur_o1 = None
                    if gi < ng:
                        blk, q = groups[gi]
                        if q == 0:
                            xtiles[blk] = xp.tile(
                                [128, S * 128], bf16, name="xbig"
                            )
                            if blk == 0:
                                # chunked so the first mm1s start sooner
                                W = (S * 128) // 4
                                for qq in range(4):
                                    xdma.dma_start(
                                        xtiles[blk][:, qq * W : (qq + 1) * W],
                                        x_d[blk][:, qq * W : (qq + 1) * W],
                                    )
                            else:
                                xdma.dma_start(xtiles[blk][:], x_d[blk])
                            ytiles[blk] = yp.tile(
                                [128, S * 128], bf16, name="ybig"
                            )
                        cur_o1 = ps1.tile([128, G * 128], f32)
                        xt = xtiles[blk]
                        for s8 in range(G):
                            s = q * G + s8
                            nc.tensor.matmul(
                                cur_o1[:, s8 * 128 : (s8 + 1) * 128],
                                xt[:, s * 128 : (s + 1) * 128],
                                wb[:],
                            )
                    if gi < ng:
                        pend.append((gi, cur_o1))
                    if gi >= LOOKAHEAD:
                        gev, pend_o1 = pend.pop(0)
                        blk2, q2 = groups[gev]
                        u8 = up.tile([128, G * 128], bf16)
                        if gev % 10 == 8:
                            # split copy: ScalarE drains the upper bank
                            # concurrently (different PSUM banks) to shed
                            # DVE load without lengthening the group chain
                            nc.vector.tensor_copy(
                                u8[:, 0:512], pend_o1[:, 0:512]
                            )
                            nc.scalar.copy(
                                u8[:, 512:1024], pend_o1[:, 512:1024]
                            )
                        else:
                            nc.vector.tensor_copy(u8[:], pend_o1[:])
                        # mm2 reuses the banks the copy just drained (WAR
                        # via u8's RAW); frees 4 banks -> 4-deep pipeline
                        o2 = pend_o1
                        for m in range(2):
                            nc.tensor.matmul(
                                o2[:, m * 512 : (m + 1) * 512],
                                wa[:],
                                u8[:, m * 512 : (m + 1) * 512],
                            )
                        nc.scalar.activation(
                            ytiles[blk2][
                                :, q2 * (G * 128) : (q2 + 1) * (G * 128)
                            ],
                            o2[:],
                            mybir.ActivationFunctionType.Gelu,
                        )
                        if blk2 == nblocks - 1:
                            # last block: per-group chunks to shrink the tail
                            W = G * 128
                            ydma.dma_start(
                                y_d[blk2][:, q2 * W : (q2 + 1) * W],
                                ytiles[blk2][:, q2 * W : (q2 + 1) * W],
                            )
                        elif q2 == ngrp_blk - 1:
                            ydma.dma_start(y_d[blk2], ytiles[blk2][:])

    nc.compile()
    return nc


def _get_nc(tpc, n_cores=N_CORES):
    key = (tpc, n_cores)
    if key not in _CACHE:
        _CACHE[key] = _build(*key)
    return _CACHE[key]


def _blockdiag2(M):
    out = np.zeros((128, 128), np.float32)
    out[:64, :64] = M
    out[64:, 64:] = M
    return out


def _make_in_maps(x, A, B, tpc, n_cores):
    import ml_dtypes

    bf = ml_dtypes.bfloat16
    quarter = tpc // 4
    S = min(S_MAX, quarter)
    nblocks = quarter // S

    x = np.asarray(x, dtype=np.float32)
    if X_INT8:
        # global absmax int8 quantization; the scale folds into B, and
        # the SWDGE x DMA upcasts int8->bf16 (small ints are exact).
        delta = float(np.abs(x).max()) / 127.0
        if delta == 0.0:
            delta = 1.0
        xq = np.clip(np.rint(x * (1.0 / delta)), -127, 127).astype(np.int8)
        wb = (_blockdiag2(np.asarray(B, np.float32)) * delta).astype(bf)
    else:
        xq = x
        wb = _blockdiag2(np.asarray(B, np.float32)).astype(bf)
    wa = _blockdiag2(np.asarray(A, np.float32)).astype(bf)

    def permute_x(xs):
        # [t, f] -> [blk, (g,i), (st,h,k)]
        v = xs.reshape(2, 2, nblocks, S, 64, 64).transpose(2, 0, 4, 3, 1, 5)
        out = v.reshape(nblocks, 128, S * 128)
        if not X_INT8:
            out = out.astype(bf)
        return np.ascontiguousarray(out)

    in_maps = []
    for c in range(n_cores):
        in_maps.append(
            {"x": permute_x(xq[c * tpc : (c + 1) * tpc]), "wb": wb, "wa": wa}
        )
    return in_maps


def _run(x, A, B, bias, tpc=TPC, trace=False):
    from concourse.bass_utils import run_bass_kernel_spmd

    n = x.shape[0]
    n_cores = n // tpc
    assert n == n_cores * tpc

    nc = _get_nc(tpc, n_cores)

    quarter = tpc // 4
    S = min(S_MAX, quarter)
    nblocks = quarter // S

    def unpermute_y(yd):
        # [blk, (h,l), (st,g,j)] -> [t, f]
        v = np.asarray(yd).reshape(nblocks, 2, 64, S, 2, 64)
        v = v.transpose(4, 1, 0, 3, 5, 2)
        return v.reshape(tpc, D).astype(np.float32)

    in_maps = _make_in_maps(x, A, B, tpc, n_cores)

    res = run_bass_kernel_spmd(
        nc, in_maps, list(range(n_cores)), trace=trace,
        trace_cores=list(range(n_cores)) if trace else None,
    )
    y = np.concatenate([unpermute_y(r["y"]) for r in res.results], axis=0)
    b = np.asarray(bias, np.float32)
    if np.any(b):
        y = y + b
    return y.astype(np.float32), res


def kernel(x, A, B, bias):
    y, _ = _run(
        np.asarray(x), np.asarray(A), np.asarray(B), np.asarray(bias)
    )
    return y


# revision 58
# speedup vs baseline: 28.6602x; 1.0024x over previous
"""Kronecker layer forward on 8 TRN2 NeuronCores.

Computes y = gelu_exact(x @ kron(B, A)) + bias for
  x [16384, 4096] f32, A [64, 64], B [64, 64], bias [4096].

Math: with x3 = x.reshape(n, 64, 64) (feature f = i*64 + k),
  u[b, j, k] = sum_i x3[b, i, k] * B[i, j]
  y[b, j*64+l] = sum_k u[b, j, k] * A[k, l]  (then gelu, +bias)

The problem is HBM-bound (~358 GB/s/core; compute is only ~2 GFLOP
per core), so precision is spent on bytes:
  - x ships as int8 (global absmax scale, folded into the B weights;
    SWDGE DMA casts int8->bf16 in flight; small ints are exact bf16)
  - y returns as bf16; compute accumulates in f32 PSUM
Measured rel err 1.28e-2 vs the 2e-2 gate (int8 quant dominates).
HBM/core: 8 MB in + 16 MB out = 24 MB -> ~67 us floor; DVE PSUM
evacuation (~76 us busy) is the modeled critical engine.

Per-core layout (tpc tokens): token t = g*tpc/2 + h*tpc/4 + blk*S + st,
supertile = 4 tokens over (g,h) in {0,1}^2. SBUF x tile per block:
  xt[p=(g,i), f=(st,h,k)] = x3[t, i, k]
Per group of G=8 supertiles, into one 2-bank PSUM tile (bufs=4):
  mm1 (data-stationary) per st: xt_st.T @ blockdiag(B,B)
      -> u[p=(h,k), f=(g,j)]
  DVE copies the group's u to SBUF bf16 (FD=1024),
  mm2 (weight-stationary): blockdiag(A,A).T @ u8 (N=512 x2) written
      back into the banks the copy just freed -> y[p=(h,l), f=(st,g,j)]
  ScalarE exact-erf Gelu PSUM->SBUF bf16 (FD=1024).
One-group software pipeline keeps PE a batch ahead of the DVE copy.
x-in on gpsimd/SWDGE (casting), y-out on sync/HWDGE; first/last block
DMAs are chunked to shrink pipeline ramp/tail. Bias (applied after
gelu) is added host-side only if nonzero.

Sharding: pure data-parallel over tokens — 2048/core, no collectives.
"""

import numpy as np

N_CORES = 8
TOKENS = 16384
D = 4096
TPC = TOKENS // N_CORES  # tokens per core
S_MAX = 16  # supertiles per block (block = 4*S tokens)
X_INT8 = True  # ship x as int8 (global absmax scale folded into B)

_CACHE = {}


def _build(tpc, n_cores, reps=1):
    import concourse.bacc as bacc
    import concourse.mybir as mybir
    import concourse.tile as tile

    f32 = mybir.dt.float32
    bf16 = mybir.dt.bfloat16

    quarter = tpc // 4          # tokens per (g,h) quadrant
    S = min(S_MAX, quarter)     # supertiles per block
    assert quarter % S == 0
    nblocks = quarter // S
    G = 8                       # supertiles per PSUM group (2 banks)
    assert S % G == 0
    ngrp_blk = S // G           # groups per block

    nc = bacc.Bacc(
        "TRN2",
        target_bir_lowering=False,
        debug=False,
        num_devices=n_cores,
    )
    i8 = mybir.dt.int8
    x_d = nc.dram_tensor(
        "x", [nblocks, 128, S * 128], i8 if X_INT8 else bf16,
        kind="ExternalInput",
    ).ap()
    # int8 x rides SWDGE (gpsimd) which casts int8->bf16 in-flight;
    # y then takes the sync HWDGE ring. bf16 x: the reverse.
    xdma = nc.gpsimd if X_INT8 else nc.sync
    ydma = nc.sync if X_INT8 else nc.gpsimd
    wb_d = nc.dram_tensor("wb", [128, 128], bf16, kind="ExternalInput").ap()
    wa_d = nc.dram_tensor("wa", [128, 128], bf16, kind="ExternalInput").ap()
    y_d = nc.dram_tensor(
        "y", [nblocks, 128, S * 128], bf16, kind="ExternalOutput"
    ).ap()

    # groups of (blk, st0, nst): block 0 starts with two G/2 prologue
    # groups so the first evacuation waits only the first DMA chunk
    groups = []
    for blk in range(nblocks):
        if blk == 0 and S >= G:
            groups += [(0, 0, G // 2), (0, G // 2, G // 2)]
            st = G
        else:
            st = 0
        while st < S:
            groups.append((blk, st, G))
            st += G
    ng = len(groups)

    with tile.TileContext(nc) as tc:
        with (
            tc.tile_pool(name="const", bufs=1) as constp,
            tc.tile_pool(name="xp", bufs=4) as xp,
            tc.tile_pool(name="up", bufs=3) as up,
            tc.tile_pool(name="yp", bufs=4) as yp,
            tc.tile_pool(name="ps1", bufs=4, space="PSUM") as ps1,
        ):
            # weights on the ACT HWDGE ring so the first x DMA (sync ring)
            # starts at t=0
            wb = constp.tile([128, 128], bf16)
            wa = constp.tile([128, 128], bf16)
            nc.sync.dma_start(wb[:], wb_d)
            nc.sync.dma_start(wa[:], wa_d)
            # dummy gelu loads the ACT table set during the first x DMA
            scratch = constp.tile([128, 128], bf16)
            nc.scalar.activation(
                scratch[:], wb[:], mybir.ActivationFunctionType.Gelu
            )

            # reps>1 re-emits the whole pipeline (idempotent: same inputs,
            # same outputs) so bench runs can difference out dispatch
            # overhead: per-rep = (t(R) - t(1)) / (R - 1).
            for _rep in range(reps):
                xtiles = {}
                ytiles = {}
                # one-group software pipeline: PE always has the next
                # group's mm1 batch queued while this group's u-copy drains.
                LOOKAHEAD = 1
                pend = []  # mm1 outputs not yet evacuated (oldest first)
                for gi in range(ng + LOOKAHEAD):
                    cur_o1 = None
                    if gi < ng:
                        blk, st0, nst = groups[gi]
                        if st0 == 0:
                            xtiles[blk] = xp.tile(
                                [128, S * 128], bf16, name="xbig"
                            )
                            if blk == 0:
                                # chunked so the first mm1s start sooner
                                W = (S * 128) // 4
                                for qq in range(4):
                                    xdma.dma_start(
                                        xtiles[blk][:, qq * W : (qq + 1) * W],
                                        x_d[blk][:, qq * W : (qq + 1) * W],
                                    )
                            else:
                                xdma.dma_start(xtiles[blk][:], x_d[blk])
                            ytiles[blk] = yp.tile(
                                [128, S * 128], bf16, name="ybig"
                            )
                        cur_o1 = ps1.tile([128, nst * 128], f32)
                        xt = xtiles[blk]
                        for s8 in range(nst):
                            s = st0 + s8
                            nc.tensor.matmul(
                                cur_o1[:, s8 * 128 : (s8 + 1) * 128],
                                xt[:, s * 128 : (s + 1) * 128],
                                wb[:],
                            )
                    if gi < ng:
                        pend.append((gi, cur_o1))
                    if gi >= LOOKAHEAD:
                        gev, pend_o1 = pend.pop(0)
                        blk2, st2, nst2 = groups[gev]
                        W2 = nst2 * 128
                        u8 = up.tile([128, G * 128], bf16, name="u8")
                        if gev % 10 == 8 and nst2 == G:
                            # split copy: ScalarE drains the upper bank
                            # concurrently (different PSUM banks) to shed
                            # DVE load without lengthening the group chain
                            nc.vector.tensor_copy(
                                u8[:, 0:512], pend_o1[:, 0:512]
                            )
                            nc.scalar.copy(
                                u8[:, 512:1024], pend_o1[:, 512:1024]
                            )
                        else:
                            nc.vector.tensor_copy(u8[:, :W2], pend_o1[:])
                        # mm2 reuses the banks the copy just drained (WAR
                        # via u8's RAW); frees banks -> deep pipeline
                        o2 = pend_o1
                        for m in range(W2 // 512):
                            nc.tensor.matmul(
                                o2[:, m * 512 : (m + 1) * 512],
                                wa[:],
                                u8[:, m * 512 : (m + 1) * 512],
                            )
                        yo = st2 * 128
                        nc.scalar.activation(
                            ytiles[blk2][:, yo : yo + W2],
                            o2[:],
                            mybir.ActivationFunctionType.Gelu,
                        )
                        if blk2 == nblocks - 1:
                            # last block: per-group chunks to shrink the tail
                            ydma.dma_start(
                                y_d[blk2][:, yo : yo + W2],
                                ytiles[blk2][:, yo : yo + W2],
                            )
                        elif st2 + nst2 == S:
                            ydma.dma_start(y_d[blk2], ytiles[blk2][:])

    nc.compile()
    return nc


def _get_nc(tpc, n_cores=N_CORES):
    key = (tpc, n_cores)
    if key not in _CACHE:
        _CACHE[key] = _build(*key)
    return _CACHE[key]


def _blockdiag2(M):
    out = np.zeros((128, 128), np.float32)
    out[:64, :64] = M
    out[64:, 64:] = M
    return out


def _make_in_maps(x, A, B, tpc, n_cores):
    import ml_dtypes

    bf = ml_dtypes.bfloat16
    quarter = tpc // 4
    S = min(S_MAX, quarter)
    nblocks = quarter // S

    x = np.asarray(x, dtype=np.float32)
    if X_INT8:
        # global absmax int8 quantization; the scale folds into B, and
        # the SWDGE x DMA upcasts int8->bf16 (small ints are exact).
        delta = float(np.abs(x).max()) / 127.0
        if delta == 0.0:
            delta = 1.0
        xq = np.clip(np.rint(x * (1.0 / delta)), -127, 127).astype(np.int8)
        wb = (_blockdiag2(np.asarray(B, np.float32)) * delta).astype(bf)
    else:
        xq = x
        wb = _blockdiag2(np.asarray(B, np.float32)).astype(bf)
    wa = _blockdiag2(np.asarray(A, np.float32)).astype(bf)

    def permute_x(xs):
        # [t, f] -> [blk, (g,i), (st,h,k)]
        v = xs.reshape(2, 2, nblocks, S, 64, 64).transpose(2, 0, 4, 3, 1, 5)
        out = v.reshape(nblocks, 128, S * 128)
        if not X_INT8:
            out = out.astype(bf)
        return np.ascontiguousarray(out)

    in_maps = []
    for c in range(n_cores):
        in_maps.append(
            {"x": permute_x(xq[c * tpc : (c + 1) * tpc]), "wb": wb, "wa": wa}
        )
    return in_maps


def _run(x, A, B, bias, tpc=TPC, trace=False):
    from concourse.bass_utils import run_bass_kernel_spmd

    n = x.shape[0]
    n_cores = n // tpc
    assert n == n_cores * tpc

    nc = _get_nc(tpc, n_cores)

    quarter = tpc // 4
    S = min(S_MAX, quarter)
    nblocks = quarter // S

    def unpermute_y(yd):
        # [blk, (h,l), (st,g,j)] -> [t, f]
        v = np.asarray(yd).reshape(nblocks, 2, 64, S, 2, 64)
        v = v.transpose(4, 1, 0, 3, 5, 2)
        return v.reshape(tpc, D).astype(np.float32)

    in_maps = _make_in_maps(x, A, B, tpc, n_cores)

    res = run_bass_kernel_spmd(
        nc, in_maps, list(range(n_cores)), trace=trace,
        trace_cores=list(range(n_cores)) if trace else None,
    )
    y = np.concatenate([unpermute_y(r["y"]) for r in res.results], axis=0)
    b = np.asarray(bias, np.float32)
    if np.any(b):
        y = y + b
    return y.astype(np.float32), res


def kernel(x, A, B, bias):
    y, _ = _run(
        np.asarray(x), np.asarray(A), np.asarray(B), np.asarray(bias)
    )
    return y


# revision 63
# speedup vs baseline: 28.7609x; 1.0035x over previous
"""Kronecker layer forward on 8 TRN2 NeuronCores.

Computes y = gelu_exact(x @ kron(B, A)) + bias for
  x [16384, 4096] f32, A [64, 64], B [64, 64], bias [4096].

Math: with x3 = x.reshape(n, 64, 64) (feature f = i*64 + k),
  u[b, j, k] = sum_i x3[b, i, k] * B[i, j]
  y[b, j*64+l] = sum_k u[b, j, k] * A[k, l]  (then gelu, +bias)

The problem is HBM-bound (~358 GB/s/core; compute is only ~2 GFLOP
per core), so precision is spent on bytes:
  - x ships as int8 (global absmax scale, folded into the B weights;
    SWDGE DMA casts int8->bf16 in flight; small ints are exact bf16)
  - y returns as bf16; compute accumulates in f32 PSUM
Measured rel err 1.28e-2 vs the 2e-2 gate (int8 quant dominates).
HBM/core: 8 MB in + 16 MB out = 24 MB -> ~67 us floor; DVE PSUM
evacuation (~76 us busy) is the modeled critical engine.

Per-core layout (tpc tokens): token t = g*tpc/2 + h*tpc/4 + blk*S + st,
supertile = 4 tokens over (g,h) in {0,1}^2. SBUF x tile per block:
  xt[p=(g,i), f=(st,h,k)] = x3[t, i, k]
Per group of G=8 supertiles, into one 2-bank PSUM tile (bufs=4):
  mm1 (data-stationary) per st: xt_st.T @ blockdiag(B,B)
      -> u[p=(h,k), f=(g,j)]
  DVE copies the group's u to SBUF bf16 (FD=1024),
  mm2 (weight-stationary): blockdiag(A,A).T @ u8 (N=512 x2) written
      back into the banks the copy just freed -> y[p=(h,l), f=(st,g,j)]
  ScalarE exact-erf Gelu PSUM->SBUF bf16 (FD=1024).
One-group software pipeline keeps PE a batch ahead of the DVE copy.
x-in on gpsimd/SWDGE (casting), y-out on sync/HWDGE; first/last block
DMAs are chunked to shrink pipeline ramp/tail. Bias (applied after
gelu) is added host-side only if nonzero.

Sharding: pure data-parallel over tokens — 2048/core, no collectives.
"""

import numpy as np

N_CORES = 8
TOKENS = 16384
D = 4096
TPC = TOKENS // N_CORES  # tokens per core
S_MAX = 16  # supertiles per block (block = 4*S tokens)
X_INT8 = True  # ship x as int8 (global absmax scale folded into B)

_CACHE = {}


def _build(tpc, n_cores, reps=1):
    import concourse.bacc as bacc
    import concourse.mybir as mybir
    import concourse.tile as tile

    f32 = mybir.dt.float32
    bf16 = mybir.dt.bfloat16

    quarter = tpc // 4          # tokens per (g,h) quadrant
    S = min(S_MAX, quarter)     # supertiles per block
    assert quarter % S == 0
    nblocks = quarter // S
    G = 8                       # supertiles per PSUM group (2 banks)
    assert S % G == 0
    ngrp_blk = S // G           # groups per block

    nc = bacc.Bacc(
        "TRN2",
        target_bir_lowering=False,
        debug=False,
        num_devices=n_cores,
    )
    i8 = mybir.dt.int8
    x_d = nc.dram_tensor(
        "x", [nblocks, 128, S * 128], i8 if X_INT8 else bf16,
        kind="ExternalInput",
    ).ap()
    # int8 x rides SWDGE (gpsimd) which casts int8->bf16 in-flight;
    # y then takes the sync HWDGE ring. bf16 x: the reverse.
    xdma = nc.gpsimd if X_INT8 else nc.sync
    ydma = nc.sync if X_INT8 else nc.gpsimd
    wb_d = nc.dram_tensor("wb", [128, 128], bf16, kind="ExternalInput").ap()
    wa_d = nc.dram_tensor("wa", [128, 128], bf16, kind="ExternalInput").ap()
    y_d = nc.dram_tensor(
        "y", [nblocks, 128, S * 128], bf16, kind="ExternalOutput"
    ).ap()

    # groups of (blk, st0, nst): block 0 starts with two G/2 prologue
    # groups so the first evacuation waits only the first DMA chunk
    groups = []
    for blk in range(nblocks):
        if blk == 0 and S >= G:
            groups += [(0, 0, G // 4), (0, G // 4, G // 4),
                       (0, G // 2, G // 2)]
            st = G
        else:
            st = 0
        while st < S:
            groups.append((blk, st, G))
            st += G
    ng = len(groups)

    with tile.TileContext(nc) as tc:
        with (
            tc.tile_pool(name="const", bufs=1) as constp,
            tc.tile_pool(name="xp", bufs=4) as xp,
            tc.tile_pool(name="up", bufs=3) as up,
            tc.tile_pool(name="yp", bufs=4) as yp,
            tc.tile_pool(name="ps1", bufs=4, space="PSUM") as ps1,
        ):
            # weights on the ACT HWDGE ring so the first x DMA (sync ring)
            # starts at t=0
            wb = constp.tile([128, 128], bf16)
            wa = constp.tile([128, 128], bf16)
            nc.sync.dma_start(wb[:], wb_d)
            nc.sync.dma_start(wa[:], wa_d)
            # dummy gelu loads the ACT table set during the first x DMA
            scratch = constp.tile([128, 128], bf16)
            nc.scalar.activation(
                scratch[:], wb[:], mybir.ActivationFunctionType.Gelu
            )

            # reps>1 re-emits the whole pipeline (idempotent: same inputs,
            # same outputs) so bench runs can difference out dispatch
            # overhead: per-rep = (t(R) - t(1)) / (R - 1).
            for _rep in range(reps):
                xtiles = {}
                ytiles = {}
                # one-group software pipeline: PE always has the next
                # group's mm1 batch queued while this group's u-copy drains.
                LOOKAHEAD = 1
                pend = []  # mm1 outputs not yet evacuated (oldest first)
                for gi in range(ng + LOOKAHEAD):
                    cur_o1 = None
                    if gi < ng:
                        blk, st0, nst = groups[gi]
                        if st0 == 0:
                            xtiles[blk] = xp.tile(
                                [128, S * 128], bf16, name="xbig"
                            )
                            if blk == 0:
                                # chunked so the first mm1s start sooner
                                W = (S * 128) // 4
                                for qq in range(4):
                                    xdma.dma_start(
                                        xtiles[blk][:, qq * W : (qq + 1) * W],
                                        x_d[blk][:, qq * W : (qq + 1) * W],
                                    )
                            else:
                                xdma.dma_start(xtiles[blk][:], x_d[blk])
                            ytiles[blk] = yp.tile(
                                [128, S * 128], bf16, name="ybig"
                            )
                        cur_o1 = ps1.tile([128, nst * 128], f32)
                        xt = xtiles[blk]
                        for s8 in range(nst):
                            s = st0 + s8
                            nc.tensor.matmul(
                                cur_o1[:, s8 * 128 : (s8 + 1) * 128],
                                xt[:, s * 128 : (s + 1) * 128],
                                wb[:],
                            )
                    if gi < ng:
                        pend.append((gi, cur_o1))
                    if gi >= LOOKAHEAD:
                        gev, pend_o1 = pend.pop(0)
                        blk2, st2, nst2 = groups[gev]
                        W2 = nst2 * 128
                        u8 = up.tile([128, G * 128], bf16, name="u8")
                        if gev % 10 == 8 and nst2 == G:
                            # split copy: ScalarE drains the upper bank
                            # concurrently (different PSUM banks) to shed
                            # DVE load without lengthening the group chain
                            nc.vector.tensor_copy(
                                u8[:, 0:512], pend_o1[:, 0:512]
                            )
                            nc.scalar.copy(
                                u8[:, 512:1024], pend_o1[:, 512:1024]
                            )
                        else:
                            nc.vector.tensor_copy(u8[:, :W2], pend_o1[:])
                        # mm2 reuses the banks the copy just drained (WAR
                        # via u8's RAW); frees banks -> deep pipeline
                        o2 = pend_o1
                        for m in range((W2 + 511) // 512):
                            N2 = min(512, W2 - m * 512)
                            nc.tensor.matmul(
                                o2[:, m * 512 : m * 512 + N2],
                                wa[:],
                                u8[:, m * 512 : m * 512 + N2],
                            )
                        yo = st2 * 128
                        nc.scalar.activation(
                            ytiles[blk2][:, yo : yo + W2],
                            o2[:],
                            mybir.ActivationFunctionType.Gelu,
                        )
                        if blk2 == nblocks - 1:
                            # last block: per-group chunks to shrink the tail
                            ydma.dma_start(
                                y_d[blk2][:, yo : yo + W2],
                                ytiles[blk2][:, yo : yo + W2],
                            )
                        elif st2 + nst2 == S:
                            ydma.dma_start(y_d[blk2], ytiles[blk2][:])

    nc.compile()
    return nc


def _get_nc(tpc, n_cores=N_CORES):
    key = (tpc, n_cores)
    if key not in _CACHE:
        _CACHE[key] = _build(*key)
    return _CACHE[key]


def _blockdiag2(M):
    out = np.zeros((128, 128), np.float32)
    out[:64, :64] = M
    out[64:, 64:] = M
    return out


def _make_in_maps(x, A, B, tpc, n_cores):
    import ml_dtypes

    bf = ml_dtypes.bfloat16
    quarter = tpc // 4
    S = min(S_MAX, quarter)
    nblocks = quarter // S

    x = np.asarray(x, dtype=np.float32)
    if X_INT8:
        # global absmax int8 quantization; the scale folds into B, and
        # the SWDGE x DMA upcasts int8->bf16 (small ints are exact).
        delta = float(np.abs(x).max()) / 127.0
        if delta == 0.0:
            delta = 1.0
        xq = np.clip(np.rint(x * (1.0 / delta)), -127, 127).astype(np.int8)
        wb = (_blockdiag2(np.asarray(B, np.float32)) * delta).astype(bf)
    else:
        xq = x
        wb = _blockdiag2(np.asarray(B, np.float32)).astype(bf)
    wa = _blockdiag2(np.asarray(A, np.float32)).astype(bf)

    def permute_x(xs):
        # [t, f] -> [blk, (g,i), (st,h,k)]
        v = xs.reshape(2, 2, nblocks, S, 64, 64).transpose(2, 0, 4, 3, 1, 5)
        out = v.reshape(nblocks, 128, S * 128)
        if not X_INT8:
            out = out.astype(bf)
        return np.ascontiguousarray(out)

    in_maps = []
    for c in range(n_cores):
        in_maps.append(
            {"x": permute_x(xq[c * tpc : (c + 1) * tpc]), "wb": wb, "wa": wa}
        )
    return in_maps


def _run(x, A, B, bias, tpc=TPC, trace=False):
    from concourse.bass_utils import run_bass_kernel_spmd

    n = x.shape[0]
    n_cores = n // tpc
    assert n == n_cores * tpc

    nc = _get_nc(tpc, n_cores)

    quarter = tpc // 4
    S = min(S_MAX, quarter)
    nblocks = quarter // S

    def unpermute_y(yd):
        # [blk, (h,l), (st,g,j)] -> [t, f]
        v = np.asarray(yd).reshape(nblocks, 2, 64, S, 2, 64)
        v = v.transpose(4, 1, 0, 3, 5, 2)
        return v.reshape(tpc, D).astype(np.float32)

    in_maps = _make_in_maps(x, A, B, tpc, n_cores)

    res = run_bass_kernel_spmd(
        nc, in_maps, list(range(n_cores)), trace=trace,
        trace_cores=list(range(n_cores)) if trace else None,
    )
    y = np.concatenate([unpermute_y(r["y"]) for r in res.results], axis=0)
    b = np.asarray(bias, np.float32)
    if np.any(b):
        y = y + b
    return y.astype(np.float32), res


def kernel(x, A, B, bias):
    y, _ = _run(
        np.asarray(x), np.asarray(A), np.asarray(B), np.asarray(bias)
    )
    return y


# revision 72
# speedup vs baseline: 28.8701x; 1.0038x over previous
"""Kronecker layer forward on 8 TRN2 NeuronCores.

Computes y = gelu_exact(x @ kron(B, A)) + bias for
  x [16384, 4096] f32, A [64, 64], B [64, 64], bias [4096].

Math: with x3 = x.reshape(n, 64, 64) (feature f = i*64 + k),
  u[b, j, k] = sum_i x3[b, i, k] * B[i, j]
  y[b, j*64+l] = sum_k u[b, j, k] * A[k, l]  (then gelu, +bias)

The problem is HBM-bound (~358 GB/s/core; compute is only ~2 GFLOP
per core), so precision is spent on bytes:
  - x ships as int8 (global absmax scale, folded into the B weights;
    SWDGE DMA casts int8->bf16 in flight; small ints are exact bf16)
  - y returns as bf16; compute accumulates in f32 PSUM
Measured rel err 1.28e-2 vs the 2e-2 gate (int8 quant dominates).
HBM/core: 8 MB in + 16 MB out = 24 MB -> ~67 us floor; DVE PSUM
evacuation (~76 us busy) is the modeled critical engine.

Per-core layout (tpc tokens): token t = g*tpc/2 + h*tpc/4 + blk*S + st,
supertile = 4 tokens over (g,h) in {0,1}^2. SBUF x tile per block:
  xt[p=(g,i), f=(st,h,k)] = x3[t, i, k]
Per group of G=8 supertiles, into one 2-bank PSUM tile (bufs=4):
  mm1 (data-stationary) per st: xt_st.T @ blockdiag(B,B)
      -> u[p=(h,k), f=(g,j)]
  DVE copies the group's u to SBUF bf16 (FD=1024),
  mm2 (weight-stationary): blockdiag(A,A).T @ u8 (N=512 x2) written
      back into the banks the copy just freed -> y[p=(h,l), f=(st,g,j)]
  ScalarE exact-erf Gelu PSUM->SBUF bf16 (FD=1024).
One-group software pipeline keeps PE a batch ahead of the DVE copy.
x-in on gpsimd/SWDGE (casting), y-out on sync/HWDGE; first/last block
DMAs are chunked to shrink pipeline ramp/tail. Bias (applied after
gelu) is added host-side only if nonzero.

Sharding: pure data-parallel over tokens — 2048/core, no collectives.
"""

import numpy as np

N_CORES = 8
TOKENS = 16384
D = 4096
TPC = TOKENS // N_CORES  # tokens per core
S_MAX = 16  # supertiles per block (block = 4*S tokens)
X_INT8 = True  # ship x as int8 (global absmax scale folded into B)

_CACHE = {}


def _build(tpc, n_cores, reps=1):
    import concourse.bacc as bacc
    import concourse.mybir as mybir
    import concourse.tile as tile

    f32 = mybir.dt.float32
    bf16 = mybir.dt.bfloat16

    quarter = tpc // 4          # tokens per (g,h) quadrant
    S = min(S_MAX, quarter)     # supertiles per block
    assert quarter % S == 0
    nblocks = quarter // S
    G = 8                       # supertiles per PSUM group (2 banks)
    assert S % G == 0
    ngrp_blk = S // G           # groups per block

    nc = bacc.Bacc(
        "TRN2",
        target_bir_lowering=False,
        debug=False,
        num_devices=n_cores,
    )
    i8 = mybir.dt.int8
    x_d = nc.dram_tensor(
        "x", [nblocks, 128, S * 128], i8 if X_INT8 else bf16,
        kind="ExternalInput",
    ).ap()
    # int8 x rides SWDGE (gpsimd) which casts int8->bf16 in-flight;
    # y then takes the sync HWDGE ring. bf16 x: the reverse.
    xdma = nc.gpsimd if X_INT8 else nc.sync
    ydma = nc.sync if X_INT8 else nc.gpsimd
    wb_d = nc.dram_tensor("wb", [128, 128], bf16, kind="ExternalInput").ap()
    wa_d = nc.dram_tensor("wa", [128, 128], bf16, kind="ExternalInput").ap()
    y_d = nc.dram_tensor(
        "y", [nblocks, 128, S * 128], bf16, kind="ExternalOutput"
    ).ap()

    # groups of (blk, st0, nst): block 0 starts with two G/2 prologue
    # groups so the first evacuation waits only the first DMA chunk
    groups = []
    for blk in range(nblocks):
        if blk == 0 and S >= G:
            groups += [(0, 0, G // 4), (0, G // 4, G // 4),
                       (0, G // 2, G // 2)]
            st = G
        else:
            st = 0
        while st < S:
            groups.append((blk, st, G))
            st += G
    ng = len(groups)

    with tile.TileContext(nc) as tc:
        with (
            tc.tile_pool(name="const", bufs=1) as constp,
            tc.tile_pool(name="xp", bufs=4) as xp,
            tc.tile_pool(name="up", bufs=3) as up,
            tc.tile_pool(name="yp", bufs=4) as yp,
            tc.tile_pool(name="ps1", bufs=4, space="PSUM") as ps1,
        ):
            # weights on the ACT HWDGE ring so the first x DMA (sync ring)
            # starts at t=0
            wb = constp.tile([128, 128], bf16)
            wa = constp.tile([128, 128], bf16)
            nc.sync.dma_start(wb[:], wb_d)
            nc.sync.dma_start(wa[:], wa_d)
            # dummy gelu loads the ACT table set during the first x DMA
            scratch = constp.tile([128, 128], bf16)
            nc.scalar.activation(
                scratch[:], wb[:], mybir.ActivationFunctionType.Gelu
            )

            # reps>1 re-emits the whole pipeline (idempotent: same inputs,
            # same outputs) so bench runs can difference out dispatch
            # overhead: per-rep = (t(R) - t(1)) / (R - 1).
            for _rep in range(reps):
                xtiles = {}
                ytiles = {}
                # one-group software pipeline: PE always has the next
                # group's mm1 batch queued while this group's u-copy drains.
                LOOKAHEAD = 1
                pend = []  # mm1 outputs not yet evacuated (oldest first)
                for gi in range(ng + LOOKAHEAD):
                    cur_o1 = None
                    if gi < ng:
                        blk, st0, nst = groups[gi]
                        if st0 == 0:
                            xtiles[blk] = xp.tile(
                                [128, S * 128], bf16, name="xbig"
                            )
                            if blk == 0:
                                # chunked so the first mm1s start sooner
                                W = (S * 128) // 4
                                for qq in range(4):
                                    xdma.dma_start(
                                        xtiles[blk][:, qq * W : (qq + 1) * W],
                                        x_d[blk][:, qq * W : (qq + 1) * W],
                                    )
                            else:
                                xdma.dma_start(xtiles[blk][:], x_d[blk])
                            ytiles[blk] = yp.tile(
                                [128, S * 128], bf16, name="ybig"
                            )
                        cur_o1 = ps1.tile([128, nst * 128], f32)
                        xt = xtiles[blk]
                        for s8 in range(nst):
                            s = st0 + s8
                            nc.tensor.matmul(
                                cur_o1[:, s8 * 128 : (s8 + 1) * 128],
                                xt[:, s * 128 : (s + 1) * 128],
                                wb[:],
                            )
                    if gi < ng:
                        pend.append((gi, cur_o1))
                    if gi >= LOOKAHEAD:
                        gev, pend_o1 = pend.pop(0)
                        blk2, st2, nst2 = groups[gev]
                        W2 = nst2 * 128
                        u8 = up.tile([128, G * 128], bf16, name="u8")
                        if gev % 10 == 6 and nst2 == G:
                            # split copy: ScalarE drains the upper bank
                            # concurrently (different PSUM banks) to shed
                            # DVE load without lengthening the group chain
                            nc.vector.tensor_copy(
                                u8[:, 0:512], pend_o1[:, 0:512]
                            )
                            nc.scalar.copy(
                                u8[:, 512:1024], pend_o1[:, 512:1024]
                            )
                        else:
                            nc.vector.tensor_copy(u8[:, :W2], pend_o1[:])
                        # mm2 reuses the banks the copy just drained (WAR
                        # via u8's RAW); frees banks -> deep pipeline
                        o2 = pend_o1
                        for m in range((W2 + 511) // 512):
                            N2 = min(512, W2 - m * 512)
                            nc.tensor.matmul(
                                o2[:, m * 512 : m * 512 + N2],
                                wa[:],
                                u8[:, m * 512 : m * 512 + N2],
                            )
                        yo = st2 * 128
                        nc.scalar.activation(
                            ytiles[blk2][:, yo : yo + W2],
                            o2[:],
                            mybir.ActivationFunctionType.Gelu,
                        )
                        if blk2 == nblocks - 1:
                            # last block: per-group chunks to shrink the tail
                            ydma.dma_start(
                                y_d[blk2][:, yo : yo + W2],
                                ytiles[blk2][:, yo : yo + W2],
                            )
                        elif st2 + nst2 == S:
                            ydma.dma_start(y_d[blk2], ytiles[blk2][:])

    nc.compile()
    return nc


def _get_nc(tpc, n_cores=N_CORES):
    key = (tpc, n_cores)
    if key not in _CACHE:
        _CACHE[key] = _build(*key)
    return _CACHE[key]


def _blockdiag2(M):
    out = np.zeros((128, 128), np.float32)
    out[:64, :64] = M
    out[64:, 64:] = M
    return out


def _make_in_maps(x, A, B, tpc, n_cores):
    import ml_dtypes

    bf = ml_dtypes.bfloat16
    quarter = tpc // 4
    S = min(S_MAX, quarter)
    nblocks = quarter // S

    x = np.asarray(x, dtype=np.float32)
    if X_INT8:
        # global absmax int8 quantization; the scale folds into B, and
        # the SWDGE x DMA upcasts int8->bf16 (small ints are exact).
        delta = float(np.abs(x).max()) / 127.0
        if delta == 0.0:
            delta = 1.0
        xq = np.clip(np.rint(x * (1.0 / delta)), -127, 127).astype(np.int8)
        wb = (_blockdiag2(np.asarray(B, np.float32)) * delta).astype(bf)
    else:
        xq = x
        wb = _blockdiag2(np.asarray(B, np.float32)).astype(bf)
    wa = _blockdiag2(np.asarray(A, np.float32)).astype(bf)

    def permute_x(xs):
        # [t, f] -> [blk, (g,i), (st,h,k)]
        v = xs.reshape(2, 2, nblocks, S, 64, 64).transpose(2, 0, 4, 3, 1, 5)
        out = v.reshape(nblocks, 128, S * 128)
        if not X_INT8:
            out = out.astype(bf)
        return np.ascontiguousarray(out)

    in_maps = []
    for c in range(n_cores):
        in_maps.append(
            {"x": permute_x(xq[c * tpc : (c + 1) * tpc]), "wb": wb, "wa": wa}
        )
    return in_maps


def _run(x, A, B, bias, tpc=TPC, trace=False):
    from concourse.bass_utils import run_bass_kernel_spmd

    n = x.shape[0]
    n_cores = n // tpc
    assert n == n_cores * tpc

    nc = _get_nc(tpc, n_cores)

    quarter = tpc // 4
    S = min(S_MAX, quarter)
    nblocks = quarter // S

    def unpermute_y(yd):
        # [blk, (h,l), (st,g,j)] -> [t, f]
        v = np.asarray(yd).reshape(nblocks, 2, 64, S, 2, 64)
        v = v.transpose(4, 1, 0, 3, 5, 2)
        return v.reshape(tpc, D).astype(np.float32)

    in_maps = _make_in_maps(x, A, B, tpc, n_cores)

    res = run_bass_kernel_spmd(
        nc, in_maps, list(range(n_cores)), trace=trace,
        trace_cores=list(range(n_cores)) if trace else None,
    )
    y = np.concatenate([unpermute_y(r["y"]) for r in res.results], axis=0)
    b = np.asarray(bias, np.float32)
    if np.any(b):
        y = y + b
    return y.astype(np.float32), res


def kernel(x, A, B, bias):
    y, _ = _run(
        np.asarray(x), np.asarray(A), np.asarray(B), np.asarray(bias)
    )
    return y


# revision 74
# speedup vs baseline: 28.8708x; 1.0000x over previous
"""Kronecker layer forward on 8 TRN2 NeuronCores.

Computes y = gelu_exact(x @ kron(B, A)) + bias for
  x [16384, 4096] f32, A [64, 64], B [64, 64], bias [4096].

Math: with x3 = x.reshape(n, 64, 64) (feature f = i*64 + k),
  u[b, j, k] = sum_i x3[b, i, k] * B[i, j]
  y[b, j*64+l] = sum_k u[b, j, k] * A[k, l]  (then gelu, +bias)

The problem is HBM-bound (~358 GB/s/core; compute is only ~2 GFLOP
per core), so precision is spent on bytes:
  - x ships as int8 (global absmax scale, folded into the B weights;
    SWDGE DMA casts int8->bf16 in flight; small ints are exact bf16)
  - y returns as bf16; compute accumulates in f32 PSUM
Measured rel err 1.28e-2 vs the 2e-2 gate (int8 quant dominates).
HBM/core: 8 MB in + 16 MB out = 24 MB -> ~67 us floor; DVE PSUM
evacuation (~76 us busy) is the modeled critical engine.

Per-core layout (tpc tokens): token t = g*tpc/2 + h*tpc/4 + blk*S + st,
supertile = 4 tokens over (g,h) in {0,1}^2. SBUF x tile per block:
  xt[p=(g,i), f=(st,h,k)] = x3[t, i, k]
Per group of G=8 supertiles, into one 2-bank PSUM tile (bufs=4):
  mm1 (data-stationary) per st: xt_st.T @ blockdiag(B,B)
      -> u[p=(h,k), f=(g,j)]
  DVE copies the group's u to SBUF bf16 (FD=1024),
  mm2 (weight-stationary): blockdiag(A,A).T @ u8 (N=512 x2) written
      back into the banks the copy just freed -> y[p=(h,l), f=(st,g,j)]
  ScalarE exact-erf Gelu PSUM->SBUF bf16 (FD=1024).
One-group software pipeline keeps PE a batch ahead of the DVE copy.
x-in on gpsimd/SWDGE (casting), y-out on sync/HWDGE; first/last block
DMAs are chunked to shrink pipeline ramp/tail. Bias (applied after
gelu) is added host-side only if nonzero.

Sharding: pure data-parallel over tokens — 2048/core, no collectives.
"""

import numpy as np

N_CORES = 8
TOKENS = 16384
D = 4096
TPC = TOKENS // N_CORES  # tokens per core
S_MAX = 16  # supertiles per block (block = 4*S tokens)
X_INT8 = True  # ship x as int8 (global absmax scale folded into B)

_CACHE = {}


def _build(tpc, n_cores, reps=1):
    import concourse.bacc as bacc
    import concourse.mybir as mybir
    import concourse.tile as tile

    f32 = mybir.dt.float32
    bf16 = mybir.dt.bfloat16

    quarter = tpc // 4          # tokens per (g,h) quadrant
    S = min(S_MAX, quarter)     # supertiles per block
    assert quarter % S == 0
    nblocks = quarter // S
    G = 8                       # supertiles per PSUM group (2 banks)
    assert S % G == 0
    ngrp_blk = S // G           # groups per block

    nc = bacc.Bacc(
        "TRN2",
        target_bir_lowering=False,
        debug=False,
        num_devices=n_cores,
    )
    i8 = mybir.dt.int8
    x_d = nc.dram_tensor(
        "x", [nblocks, 128, S * 128], i8 if X_INT8 else bf16,
        kind="ExternalInput",
    ).ap()
    # int8 x rides SWDGE (gpsimd) which casts int8->bf16 in-flight;
    # y then takes the sync HWDGE ring. bf16 x: the reverse.
    xdma = nc.gpsimd if X_INT8 else nc.sync
    ydma = nc.sync if X_INT8 else nc.gpsimd
    wb_d = nc.dram_tensor("wb", [128, 128], bf16, kind="ExternalInput").ap()
    wa_d = nc.dram_tensor("wa", [128, 128], bf16, kind="ExternalInput").ap()
    y_d = nc.dram_tensor(
        "y", [nblocks, 128, S * 128], bf16, kind="ExternalOutput"
    ).ap()

    # groups of (blk, st0, nst): block 0 starts with two G/2 prologue
    # groups so the first evacuation waits only the first DMA chunk
    groups = []
    for blk in range(nblocks):
        if blk == 0 and S >= G:
            groups += [(0, 0, G // 4), (0, G // 4, G // 4),
                       (0, G // 2, G // 2)]
            st = G
        else:
            st = 0
        while st < S:
            groups.append((blk, st, G))
            st += G
    ng = len(groups)

    with tile.TileContext(nc) as tc:
        with (
            tc.tile_pool(name="const", bufs=1) as constp,
            tc.tile_pool(name="xp", bufs=4) as xp,
            tc.tile_pool(name="up", bufs=3) as up,
            tc.tile_pool(name="yp", bufs=4) as yp,
            tc.tile_pool(name="ps1", bufs=4, space="PSUM") as ps1,
        ):
            # weights on the ACT HWDGE ring so the first x DMA (sync ring)
            # starts at t=0
            wb = constp.tile([128, 128], bf16)
            wa = constp.tile([128, 128], bf16)
            nc.sync.dma_start(wb[:], wb_d)
            nc.sync.dma_start(wa[:], wa_d)
            # dummy gelu loads the ACT table set during the first x DMA
            scratch = constp.tile([128, 128], bf16)
            nc.scalar.activation(
                scratch[:], wb[:], mybir.ActivationFunctionType.Gelu
            )

            # reps>1 re-emits the whole pipeline (idempotent: same inputs,
            # same outputs) so bench runs can difference out dispatch
            # overhead: per-rep = (t(R) - t(1)) / (R - 1).
            for _rep in range(reps):
                xtiles = {}
                ytiles = {}
                # one-group software pipeline: PE always has the next
                # group's mm1 batch queued while this group's u-copy drains.
                LOOKAHEAD = 1
                pend = []  # mm1 outputs not yet evacuated (oldest first)
                for gi in range(ng + LOOKAHEAD):
                    cur_o1 = None
                    if gi < ng:
                        blk, st0, nst = groups[gi]
                        if st0 == 0:
                            xtiles[blk] = xp.tile(
                                [128, S * 128], bf16, name="xbig"
                            )
                            if blk == 0:
                                # chunked so the first mm1s start sooner
                                W = (S * 128) // 4
                                for qq in range(4):
                                    xdma.dma_start(
                                        xtiles[blk][:, qq * W : (qq + 1) * W],
                                        x_d[blk][:, qq * W : (qq + 1) * W],
                                    )
                            else:
                                xdma.dma_start(xtiles[blk][:], x_d[blk])
                            ytiles[blk] = yp.tile(
                                [128, S * 128], bf16, name="ybig"
                            )
                        cur_o1 = ps1.tile([128, nst * 128], f32)
                        xt = xtiles[blk]
                        for s8 in range(nst):
                            s = st0 + s8
                            nc.tensor.matmul(
                                cur_o1[:, s8 * 128 : (s8 + 1) * 128],
                                xt[:, s * 128 : (s + 1) * 128],
                                wb[:],
                            )
                    if gi < ng:
                        pend.append((gi, cur_o1))
                    if gi >= LOOKAHEAD:
                        gev, pend_o1 = pend.pop(0)
                        blk2, st2, nst2 = groups[gev]
                        W2 = nst2 * 128
                        u8 = up.tile([128, G * 128], bf16, name="u8")
                        if gev % 9 == 6 and nst2 == G:
                            # split copy: ScalarE drains the upper bank
                            # concurrently (different PSUM banks) to shed
                            # DVE load without lengthening the group chain
                            nc.vector.tensor_copy(
                                u8[:, 0:512], pend_o1[:, 0:512]
                            )
                            nc.scalar.copy(
                                u8[:, 512:1024], pend_o1[:, 512:1024]
                            )
                        else:
                            nc.vector.tensor_copy(u8[:, :W2], pend_o1[:])
                        # mm2 reuses the banks the copy just drained (WAR
                        # via u8's RAW); frees banks -> deep pipeline
                        o2 = pend_o1
                        for m in range((W2 + 511) // 512):
                            N2 = min(512, W2 - m * 512)
                            nc.tensor.matmul(
                                o2[:, m * 512 : m * 512 + N2],
                                wa[:],
                                u8[:, m * 512 : m * 512 + N2],
                            )
                        yo = st2 * 128
                        nc.scalar.activation(
                            ytiles[blk2][:, yo : yo + W2],
                            o2[:],
                            mybir.ActivationFunctionType.Gelu,
                        )
                        if blk2 == nblocks - 1:
                            # last block: per-group chunks to shrink the tail
                            ydma.dma_start(
                                y_d[blk2][:, yo : yo + W2],
                                ytiles[blk2][:, yo : yo + W2],
                            )
                        elif st2 + nst2 == S:
                            ydma.dma_start(y_d[blk2], ytiles[blk2][:])

    nc.compile()
    return nc


def _get_nc(tpc, n_cores=N_CORES):
    key = (tpc, n_cores)
    if key not in _CACHE:
        _CACHE[key] = _build(*key)
    return _CACHE[key]


def _blockdiag2(M):
    out = np.zeros((128, 128), np.float32)
    out[:64, :64] = M
    out[64:, 64:] = M
    return out


def _make_in_maps(x, A, B, tpc, n_cores):
    import ml_dtypes

    bf = ml_dtypes.bfloat16
    quarter = tpc // 4
    S = min(S_MAX, quarter)
    nblocks = quarter // S

    x = np.asarray(x, dtype=np.float32)
    if X_INT8:
        # global absmax int8 quantization; the scale folds into B, and
        # the SWDGE x DMA upcasts int8->bf16 (small ints are exact).
        delta = float(np.abs(x).max()) / 127.0
        if delta == 0.0:
            delta = 1.0
        xq = np.clip(np.rint(x * (1.0 / delta)), -127, 127).astype(np.int8)
        wb = (_blockdiag2(np.asarray(B, np.float32)) * delta).astype(bf)
    else:
        xq = x
        wb = _blockdiag2(np.asarray(B, np.float32)).astype(bf)
    wa = _blockdiag2(np.asarray(A, np.float32)).astype(bf)

    def permute_x(xs):
        # [t, f] -> [blk, (g,i), (st,h,k)]
        v = xs.reshape(2, 2, nblocks, S, 64, 64).transpose(2, 0, 4, 3, 1, 5)
        out = v.reshape(nblocks, 128, S * 128)
        if not X_INT8:
            out = out.astype(bf)
        return np.ascontiguousarray(out)

    in_maps = []
    for c in range(n_cores):
        in_maps.append(
            {"x": permute_x(xq[c * tpc : (c + 1) * tpc]), "wb": wb, "wa": wa}
        )
    return in_maps


def _run(x, A, B, bias, tpc=TPC, trace=False):
    from concourse.bass_utils import run_bass_kernel_spmd

    n = x.shape[0]
    n_cores = n // tpc
    assert n == n_cores * tpc

    nc = _get_nc(tpc, n_cores)

    quarter = tpc // 4
    S = min(S_MAX, quarter)
    nblocks = quarter // S

    def unpermute_y(yd):
        # [blk, (h,l), (st,g,j)] -> [t, f]
        v = np.asarray(yd).reshape(nblocks, 2, 64, S, 2, 64)
        v = v.transpose(4, 1, 0, 3, 5, 2)
        return v.reshape(tpc, D).astype(np.float32)

    in_maps = _make_in_maps(x, A, B, tpc, n_cores)

    res = run_bass_kernel_spmd(
        nc, in_maps, list(range(n_cores)), trace=trace,
        trace_cores=list(range(n_cores)) if trace else None,
    )
    y = np.concatenate([unpermute_y(r["y"]) for r in res.results], axis=0)
    b = np.asarray(bias, np.float32)
    if np.any(b):
        y = y + b
    return y.astype(np.float32), res


def kernel(x, A, B, bias):
    y, _ = _run(
        np.asarray(x), np.asarray(A), np.asarray(B), np.asarray(bias)
    )
    return y


# revision 75
# speedup vs baseline: 28.8850x; 1.0005x over previous
"""Kronecker layer forward on 8 TRN2 NeuronCores.

Computes y = gelu_exact(x @ kron(B, A)) + bias for
  x [16384, 4096] f32, A [64, 64], B [64, 64], bias [4096].

Math: with x3 = x.reshape(n, 64, 64) (feature f = i*64 + k),
  u[b, j, k] = sum_i x3[b, i, k] * B[i, j]
  y[b, j*64+l] = sum_k u[b, j, k] * A[k, l]  (then gelu, +bias)

The problem is HBM-bound (~358 GB/s/core; compute is only ~2 GFLOP
per core), so precision is spent on bytes:
  - x ships as int8 (global absmax scale, folded into the B weights;
    SWDGE DMA casts int8->bf16 in flight; small ints are exact bf16)
  - y returns as bf16; compute accumulates in f32 PSUM
Measured rel err 1.28e-2 vs the 2e-2 gate (int8 quant dominates).
HBM/core: 8 MB in + 16 MB out = 24 MB -> ~67 us floor; DVE PSUM
evacuation (~76 us busy) is the modeled critical engine.

Per-core layout (tpc tokens): token t = g*tpc/2 + h*tpc/4 + blk*S + st,
supertile = 4 tokens over (g,h) in {0,1}^2. SBUF x tile per block:
  xt[p=(g,i), f=(st,h,k)] = x3[t, i, k]
Per group of G=8 supertiles, into one 2-bank PSUM tile (bufs=4):
  mm1 (data-stationary) per st: xt_st.T @ blockdiag(B,B)
      -> u[p=(h,k), f=(g,j)]
  DVE copies the group's u to SBUF bf16 (FD=1024),
  mm2 (weight-stationary): blockdiag(A,A).T @ u8 (N=512 x2) written
      back into the banks the copy just freed -> y[p=(h,l), f=(st,g,j)]
  ScalarE exact-erf Gelu PSUM->SBUF bf16 (FD=1024).
One-group software pipeline keeps PE a batch ahead of the DVE copy.
x-in on gpsimd/SWDGE (casting), y-out on sync/HWDGE; first/last block
DMAs are chunked to shrink pipeline ramp/tail. Bias (applied after
gelu) is added host-side only if nonzero.

Sharding: pure data-parallel over tokens — 2048/core, no collectives.
"""

import numpy as np

N_CORES = 8
TOKENS = 16384
D = 4096
TPC = TOKENS // N_CORES  # tokens per core
S_MAX = 16  # supertiles per block (block = 4*S tokens)
X_INT8 = True  # ship x as int8 (global absmax scale folded into B)

_CACHE = {}


def _build(tpc, n_cores, reps=1):
    import concourse.bacc as bacc
    import concourse.mybir as mybir
    import concourse.tile as tile

    f32 = mybir.dt.float32
    bf16 = mybir.dt.bfloat16

    quarter = tpc // 4          # tokens per (g,h) quadrant
    S = min(S_MAX, quarter)     # supertiles per block
    assert quarter % S == 0
    nblocks = quarter // S
    G = 8                       # supertiles per PSUM group (2 banks)
    assert S % G == 0
    ngrp_blk = S // G           # groups per block

    nc = bacc.Bacc(
        "TRN2",
        target_bir_lowering=False,
        debug=False,
        num_devices=n_cores,
    )
    i8 = mybir.dt.int8
    x_d = nc.dram_tensor(
        "x", [nblocks, 128, S * 128], i8 if X_INT8 else bf16,
        kind="ExternalInput",
    ).ap()
    # int8 x rides SWDGE (gpsimd) which casts int8->bf16 in-flight;
    # y then takes the sync HWDGE ring. bf16 x: the reverse.
    xdma = nc.gpsimd if X_INT8 else nc.sync
    ydma = nc.sync if X_INT8 else nc.gpsimd
    wb_d = nc.dram_tensor("wb", [128, 128], bf16, kind="ExternalInput").ap()
    wa_d = nc.dram_tensor("wa", [128, 128], bf16, kind="ExternalInput").ap()
    y_d = nc.dram_tensor(
        "y", [nblocks, 128, S * 128], bf16, kind="ExternalOutput"
    ).ap()

    # groups of (blk, st0, nst): block 0 starts with two G/2 prologue
    # groups so the first evacuation waits only the first DMA chunk
    groups = []
    for blk in range(nblocks):
        if blk == 0 and S >= G:
            groups += [(0, 0, G // 4), (0, G // 4, G // 4),
                       (0, G // 2, G // 2)]
            st = G
        else:
            st = 0
        while st < S:
            groups.append((blk, st, G))
            st += G
    ng = len(groups)

    with tile.TileContext(nc) as tc:
        with (
            tc.tile_pool(name="const", bufs=1) as constp,
            tc.tile_pool(name="xp", bufs=4) as xp,
            tc.tile_pool(name="up", bufs=3) as up,
            tc.tile_pool(name="yp", bufs=4) as yp,
            tc.tile_pool(name="ps1", bufs=4, space="PSUM") as ps1,
        ):
            # weights on the ACT HWDGE ring so the first x DMA (sync ring)
            # starts at t=0
            wb = constp.tile([128, 128], bf16)
            wa = constp.tile([128, 128], bf16)
            nc.sync.dma_start(wb[:], wb_d)
            nc.sync.dma_start(wa[:], wa_d)
            # dummy gelu loads the ACT table set during the first x DMA
            scratch = constp.tile([128, 128], bf16)
            nc.scalar.activation(
                scratch[:], wb[:], mybir.ActivationFunctionType.Gelu
            )

            # reps>1 re-emits the whole pipeline (idempotent: same inputs,
            # same outputs) so bench runs can difference out dispatch
            # overhead: per-rep = (t(R) - t(1)) / (R - 1).
            for _rep in range(reps):
                xtiles = {}
                ytiles = {}
                # one-group software pipeline: PE always has the next
                # group's mm1 batch queued while this group's u-copy drains.
                LOOKAHEAD = 1
                pend = []  # mm1 outputs not yet evacuated (oldest first)
                for gi in range(ng + LOOKAHEAD):
                    cur_o1 = None
                    if gi < ng:
                        blk, st0, nst = groups[gi]
                        if st0 == 0:
                            xtiles[blk] = xp.tile(
                                [128, S * 128], bf16, name="xbig"
                            )
                            if blk == 0:
                                # chunked so the first mm1s start sooner
                                W = (S * 128) // 4
                                for qq in range(4):
                                    xdma.dma_start(
                                        xtiles[blk][:, qq * W : (qq + 1) * W],
                                        x_d[blk][:, qq * W : (qq + 1) * W],
                                    )
                            else:
                                xdma.dma_start(xtiles[blk][:], x_d[blk])
                            ytiles[blk] = yp.tile(
                                [128, S * 128], bf16, name="ybig"
                            )
                        cur_o1 = ps1.tile([128, nst * 128], f32)
                        xt = xtiles[blk]
                        for s8 in range(nst):
                            s = st0 + s8
                            nc.tensor.matmul(
                                cur_o1[:, s8 * 128 : (s8 + 1) * 128],
                                xt[:, s * 128 : (s + 1) * 128],
                                wb[:],
                            )
                    if gi < ng:
                        pend.append((gi, cur_o1))
                    if gi >= LOOKAHEAD:
                        gev, pend_o1 = pend.pop(0)
                        blk2, st2, nst2 = groups[gev]
                        W2 = nst2 * 128
                        u8 = up.tile([128, G * 128], bf16, name="u8")
                        if gev % 9 == 5 and nst2 == G:
                            # split copy: ScalarE drains the upper bank
                            # concurrently (different PSUM banks) to shed
                            # DVE load without lengthening the group chain
                            nc.vector.tensor_copy(
                                u8[:, 0:512], pend_o1[:, 0:512]
                            )
                            nc.scalar.copy(
                                u8[:, 512:1024], pend_o1[:, 512:1024]
                            )
                        else:
                            nc.vector.tensor_copy(u8[:, :W2], pend_o1[:])
                        # mm2 reuses the banks the copy just drained (WAR
                        # via u8's RAW); frees banks -> deep pipeline
                        o2 = pend_o1
                        for m in range((W2 + 511) // 512):
                            N2 = min(512, W2 - m * 512)
                            nc.tensor.matmul(
                                o2[:, m * 512 : m * 512 + N2],
                                wa[:],
                                u8[:, m * 512 : m * 512 + N2],
                            )
                        yo = st2 * 128
                        nc.scalar.activation(
                            ytiles[blk2][:, yo : yo + W2],
                            o2[:],
                            mybir.ActivationFunctionType.Gelu,
                        )
                        if blk2 == nblocks - 1:
                            # last block: per-group chunks to shrink the tail
                            ydma.dma_start(
                                y_d[blk2][:, yo : yo + W2],
                                ytiles[blk2][:, yo : yo + W2],
                            )
                        elif st2 + nst2 == S:
                            ydma.dma_start(y_d[blk2], ytiles[blk2][:])

    nc.compile()
    return nc


def _get_nc(tpc, n_cores=N_CORES):
    key = (tpc, n_cores)
    if key not in _CACHE:
        _CACHE[key] = _build(*key)
    return _CACHE[key]


def _blockdiag2(M):
    out = np.zeros((128, 128), np.float32)
    out[:64, :64] = M
    out[64:, 64:] = M
    return out


def _make_in_maps(x, A, B, tpc, n_cores):
    import ml_dtypes

    bf = ml_dtypes.bfloat16
    quarter = tpc // 4
    S = min(S_MAX, quarter)
    nblocks = quarter // S

    x = np.asarray(x, dtype=np.float32)
    if X_INT8:
        # global absmax int8 quantization; the scale folds into B, and
        # the SWDGE x DMA upcasts int8->bf16 (small ints are exact).
        delta = float(np.abs(x).max()) / 127.0
        if delta == 0.0:
            delta = 1.0
        xq = np.clip(np.rint(x * (1.0 / delta)), -127, 127).astype(np.int8)
        wb = (_blockdiag2(np.asarray(B, np.float32)) * delta).astype(bf)
    else:
        xq = x
        wb = _blockdiag2(np.asarray(B, np.float32)).astype(bf)
    wa = _blockdiag2(np.asarray(A, np.float32)).astype(bf)

    def permute_x(xs):
        # [t, f] -> [blk, (g,i), (st,h,k)]
        v = xs.reshape(2, 2, nblocks, S, 64, 64).transpose(2, 0, 4, 3, 1, 5)
        out = v.reshape(nblocks, 128, S * 128)
        if not X_INT8:
            out = out.astype(bf)
        return np.ascontiguousarray(out)

    in_maps = []
    for c in range(n_cores):
        in_maps.append(
            {"x": permute_x(xq[c * tpc : (c + 1) * tpc]), "wb": wb, "wa": wa}
        )
    return in_maps


def _run(x, A, B, bias, tpc=TPC, trace=False):
    from concourse.bass_utils import run_bass_kernel_spmd

    n = x.shape[0]
    n_cores = n // tpc
    assert n == n_cores * tpc

    nc = _get_nc(tpc, n_cores)

    quarter = tpc // 4
    S = min(S_MAX, quarter)
    nblocks = quarter // S

    def unpermute_y(yd):
        # [blk, (h,l), (st,g,j)] -> [t, f]
        v = np.asarray(yd).reshape(nblocks, 2, 64, S, 2, 64)
        v = v.transpose(4, 1, 0, 3, 5, 2)
        return v.reshape(tpc, D).astype(np.float32)

    in_maps = _make_in_maps(x, A, B, tpc, n_cores)

    res = run_bass_kernel_spmd(
        nc, in_maps, list(range(n_cores)), trace=trace,
        trace_cores=list(range(n_cores)) if trace else None,
    )
    y = np.concatenate([unpermute_y(r["y"]) for r in res.results], axis=0)
    b = np.asarray(bias, np.float32)
    if np.any(b):
        y = y + b
    return y.astype(np.float32), res


def kernel(x, A, B, bias):
    y, _ = _run(
        np.asarray(x), np.asarray(A), np.asarray(B), np.asarray(bias)
    )
    return y
